# revision 1
# baseline (speedup 1.0000x reference)
"""Fused LayerNorm + single-head self-attention kernel for Trainium2 (8 NeuronCores).

Problem: x[4,64,64,128] -> LN(ch) -> QKV proj -> softmax(QK^T/sqrt(C)) V -> out proj.

Sharding: 2 cores per batch element. Each core computes its batch's full K/V
(4096 tokens) and one half of the queries (2048 rows). Per-core inputs differ
(x = its batch, xq = its query half) so the SPMD program is uniform and no
collectives are needed.

Host folds gamma/beta and the 1/sqrt(C) softmax scale into the projection
weights, so the device LN is just (x-mu)*rstd.

Device dataflow (per core):
  - LN via bn_stats/bn_aggr, apply as one dual-scalar tensor_scalar -> nx (bf16)
  - transpose nx tiles via PE identity-matmul -> nxT [c, tokens]
  - kT/qT = W.T @ nxT (bf16 matmuls, N=1024), v = nxT_tile.T @ Wv -> [tok, c]
  - v stored with a ones column [tok, 129] so attn@v also yields softmax denominators
  - scores(T) [tk,tq] = kT_tile.T @ qT_block in PSUM [128,1024]; exp on ScalarE
    directly PSUM->SBUF (bf16), no max-subtraction (scores bounded ~ +-11.4)
  - attn@v: accumulate over 32 tk tiles into PSUM [tq=128, 129]; col 128 = denom
  - normalize, PE-transpose, project with Wo, add bo, DMA out
"""

import os
import sys
from contextlib import ExitStack

import numpy as np

for _p in ("/opt/trn_rl_repo", "/root/.axon_site/_ro/trn_rl_repo"):
    if os.path.isdir(_p) and _p not in sys.path:
        sys.path.insert(0, _p)

import concourse.bass as bass
import concourse.tile as tile
from concourse import bacc, mybir
from concourse.bass import ds, ts
from concourse._compat import with_exitstack
from concourse.bass_utils import run_bass_kernel_spmd

B, HH, WW, C = 4, 64, 64, 128
S = HH * WW  # 4096 tokens per batch
SQ = S // 2  # 2048 query rows per core
P = 128
NT = S // P  # 32 kv token tiles
QBLK = 1024
NBLK = SQ // QBLK  # 2 query blocks per core
NTA = 24  # attn@v A-half depth (B-half = NT-NTA runs in the tail)
EPS = 1e-5

F32 = mybir.dt.float32
BF16 = mybir.dt.bfloat16


@with_exitstack
def _attention_kernel(ctx: ExitStack, tc: tile.TileContext, aps: dict):
    nc = tc.nc
    x, xq, out = aps["x"], aps["xq"], aps["out"]

    consts = ctx.enter_context(tc.tile_pool(name="consts", bufs=1))
    bigp = ctx.enter_context(tc.tile_pool(name="big", bufs=1))
    lnp = ctx.enter_context(tc.tile_pool(name="ln", bufs=3))
    statp = ctx.enter_context(tc.tile_pool(name="stat", bufs=3))
    nxp = ctx.enter_context(tc.tile_pool(name="nx", bufs=6))
    expp = ctx.enter_context(tc.tile_pool(name="expp", bufs=2))
    aop = ctx.enter_context(tc.tile_pool(name="aop", bufs=4))
    smallp = ctx.enter_context(tc.tile_pool(name="smallp", bufs=4))
    outp = ctx.enter_context(tc.tile_pool(name="outp", bufs=4))
    sap = ctx.enter_context(tc.tile_pool(name="sap", bufs=9))
    # Unified PSUM plan, coexisting all-kernel (8 banks total):
    #   U1: [128,1024] x2 bufs = 4 banks  (kq projections + scores)
    #   U2: [128,4,128] x2 bufs = 2 banks (transposes, v proj, attnv tail)
    #   U3: [128,129] x2 bufs = 2 banks   (attnv accumulators)
    u1 = ctx.enter_context(tc.tile_pool(name="u1", bufs=2, space="PSUM"))
    u2 = ctx.enter_context(tc.tile_pool(name="u2", bufs=2, space="PSUM"))
    u3 = ctx.enter_context(tc.tile_pool(name="u3", bufs=2, space="PSUM"))

    # --- constants -> SBUF, cast weights to bf16
    w_b = {}
    for name in ("wq", "wk", "wv", "wo"):
        wf = consts.tile([C, C], F32, tag=f"{name}_f")
        nc.sync.dma_start(out=wf, in_=aps[name])
        wb = consts.tile([C, C], BF16, tag=f"{name}_b")
        nc.vector.tensor_copy(wb, wf)
        w_b[name] = wb
    bq_s = consts.tile([C, 1], F32, tag="bq")
    nc.sync.dma_start(out=bq_s, in_=aps["bq"])
    bob_s = consts.tile([P, C], F32, tag="bob")
    nc.sync.dma_start(out=bob_s, in_=aps["bob"])
    idf = consts.tile([P, P], F32, tag="idf")
    nc.sync.dma_start(out=idf, in_=aps["ident"])
    id_b = consts.tile([P, P], BF16, tag="idb")
    nc.vector.tensor_copy(id_b, idf)
    eps_t = consts.tile([P, 1], F32, tag="eps")
    nc.vector.memset(eps_t, EPS)

    # --- big persistent SBUF tensors
    nxT = bigp.tile([P, S], BF16, tag="nxT")      # normalized x, transposed
    nxqT = bigp.tile([P, SQ], BF16, tag="nxqT")
    kT = bigp.tile([P, S], BF16, tag="kT")
    qT = bigp.tile([P, SQ], BF16, tag="qT")
    vsb = bigp.tile([P, NT, 130], BF16, tag="vsb")  # [tok, c] + ones col at 128
    nc.vector.memset(vsb[:, :, 128:129], 1.0)
    eTs = []
    for _bi in range(NBLK):
        eT_blk = expp.tile([P, NT, QBLK], BF16, tag="eT")
        eTs.append(eT_blk)

    def ln_group(src, g, dstT, act_copies=False, half_hook=None):
        # LayerNorm 8 token-tiles and PE-transpose them into dstT columns.
        # Processed in 4-tile halves end-to-end (stats -> rstd -> apply ->
        # transpose -> copy -> hook) so downstream work (kT chunks, scores)
        # can start after half a group instead of a full one.
        xg = lnp.tile([P, 8, C], F32, tag="xg")
        # split the 512KB load in halves on different DMA queues: half-0's
        # stats start after 256KB, and the halves transfer concurrently
        for h, eng in ((0, nc.sync), (1, nc.gpsimd)):
            eng.dma_start(
                out=xg[:, 4 * h:4 * h + 4, :],
                in_=src[(g * 8 + 4 * h) * P:(g * 8 + 4 * h + 4) * P, :]
                .rearrange("(i p) c -> p i c", p=P),
            )
        st = statp.tile([P, 8, 6], F32, tag="st")
        mv = statp.tile([P, 8, 2], F32, tag="mv")
        rstd = statp.tile([P, 8], F32, tag="rstd")
        for half in range(2):
            sl = slice(4 * half, 4 * half + 4)
            for i in range(4 * half, 4 * half + 4):
                nc.vector.bn_stats(st[:, i, :], xg[:, i, :])
                nc.vector.bn_aggr(mv[:, i, :], st[:, i, :])
            # rstd = exp(-0.5*ln(var+eps)); Ln and Exp share one activation
            # table set, so LN interleaved with exp never reloads tables
            nc.scalar.activation(
                rstd[:, sl], mv[:, sl, 1],
                func=mybir.ActivationFunctionType.Ln,
                bias=eps_t, scale=1.0)
            nc.scalar.activation(
                rstd[:, sl], rstd[:, sl],
                func=mybir.ActivationFunctionType.Exp,
                scale=-0.5)
            tp = u2.tile([P, 4, P], F32, tag="u2")
            for i in range(4 * half, 4 * half + 4):
                nxt = nxp.tile([P, C], BF16, tag="nxt")
                nc.vector.tensor_scalar(
                    nxt, xg[:, i, :], mv[:, i, 0:1], rstd[:, i:i + 1],
                    mybir.AluOpType.subtract, mybir.AluOpType.mult)
                nc.tensor.matmul(tp[:, i % 4, :], lhsT=nxt, rhs=id_b,
                                 start=True, stop=True)
            base = (g * 8 + 4 * half) * P
            if act_copies:
                # head-only: ACT is idle before the first exp; keep the
                # critical LN chain off DVE
                nc.scalar.copy(dstT[:, ds(base, 4 * P)], tp)
            else:
                nc.vector.tensor_copy(dstT[:, ds(base, 4 * P)], tp)
            if half_hook is not None:
                half_hook(half)

    # --- queries: only block-0's quarter first so attention starts early
    def emit_qproj(j):
        qp = u1.tile([P, QBLK], F32, tag="u1")
        for h in range(2):
            nc.tensor.matmul(qp[:, ts(h, 512)], lhsT=w_b["wq"],
                             rhs=nxqT[:, ds(j * QBLK + h * 512, 512)],
                             start=True, stop=True)
        nc.vector.tensor_scalar(
            qT[:, ts(j, QBLK)], qp, bq_s, None, mybir.AluOpType.add)

    ln_group(xq, 0, nxqT, act_copies=True)
    emit_qproj(0)

    # --- attention helpers
    def emit_scores(b, i):
        sp = u1.tile([P, QBLK], F32, tag="u1")
        for h in range(2):
            nc.tensor.matmul(sp[:, ts(h, 512)], lhsT=kT[:, ts(i, P)],
                             rhs=qT[:, ds(b * QBLK + h * 512, 512)],
                             start=True, stop=True)
        nc.scalar.activation(eTs[b][:, i, :], sp,
                             func=mybir.ActivationFunctionType.Exp)

    # attn@v for query sub-tile j, split in tk halves: the A half only needs
    # the first 16 exp tiles, so it runs inside the block's exp window and
    # keeps PE busy; a DVE add recombines num/den before normalization.
    sA_tiles = {}

    def emit_attnv_half_a(b, j):
        eT = eTs[b]
        op_ = u3.tile([P, 129], F32, tag="u3")
        for i in range(NTA):
            nc.tensor.matmul(op_, lhsT=eT[:, i, ts(j, P)],
                             rhs=vsb[:, i, 0:129],
                             start=(i == 0), stop=(i == NTA - 1))
        sA = sap.tile([P, 129], F32, tag="sA")
        nc.vector.tensor_copy(sA, op_)
        sA_tiles[(b, j)] = sA

    def emit_attnv_half_b(b, j):
        eT = eTs[b]
        if b == NBLK - 1 and j % 2 == 1:
            # tail block: scores are done, u1's banks are free; alternating
            # pools doubles the number of in-flight output chains
            op_ = u1.tile([P, 129], F32, tag="u1")
        else:
            op_ = u3.tile([P, 129], F32, tag="u3")
        for i in range(NTA, NT):
            nc.tensor.matmul(op_, lhsT=eT[:, i, ts(j, P)],
                             rhs=vsb[:, i, 0:129],
                             start=(i == NTA), stop=(i == NT - 1))
        tot = aop.tile([P, 129], F32, tag="tot")
        nc.vector.tensor_add(tot, op_, sA_tiles.pop((b, j)))
        # normalize AFTER the Wo projection (row scale commutes with matmul):
        # the reciprocal runs concurrently with transpose+Wo instead of
        # gating them
        ao = aop.tile([P, C], BF16, tag="ao")
        nc.vector.tensor_copy(ao, tot[:, 0:128])
        r = smallp.tile([P, 1], F32, tag="r")
        nc.vector.reciprocal(r, tot[:, 128:129])
        tfp = u2.tile([P, 4, C], F32, tag="u2")
        nc.tensor.matmul(tfp[:, 0, :], lhsT=ao, rhs=id_b,
                         start=True, stop=True)
        aoT = aop.tile([P, C], BF16, tag="aoT")
        if b == NBLK - 1:
            nc.scalar.copy(aoT, tfp[:, 0, :])  # ACT is idle in the tail
        else:
            nc.vector.tensor_copy(aoT, tfp[:, 0, :])
        nc.tensor.matmul(tfp[:, 1, :], lhsT=aoT, rhs=w_b["wo"],
                         start=True, stop=True)
        ot = outp.tile([P, C], F32, tag="ot")
        nc.vector.scalar_tensor_tensor(
            ot, tfp[:, 1, :], r, bob_s,
            mybir.AluOpType.mult, mybir.AluOpType.add)
        # alternate DMA queues so output writes drain in parallel
        eng = nc.sync if j % 2 == 0 else nc.gpsimd
        eng.dma_start(out=out[ds(b * QBLK + j * P, P), :], in_=ot)

    # keys/values + block-0 scores, emitted per half-LN-group so the exp
    # stream ignites as early as possible and never waits on later LN work
    def kv_half(g, half):
        base = g * 8 + 4 * half
        kp = u1.tile([P, 512], F32, tag="u1")
        nc.tensor.matmul(kp, lhsT=w_b["wk"], rhs=nxT[:, ds(base * P, 512)],
                         start=True, stop=True)
        if g == 0:
            nc.scalar.copy(kT[:, ds(base * P, 512)], kp)
        else:
            nc.vector.tensor_copy(kT[:, ds(base * P, 512)], kp)
        vp = u2.tile([P, 4, C], F32, tag="u2")
        for i in range(4):
            nc.tensor.matmul(vp[:, i, :], lhsT=nxT[:, ts(base + i, P)],
                             rhs=w_b["wv"], start=True, stop=True)
        nc.vector.tensor_copy(vsb[:, ds(base, 4), 0:128], vp)
        for i in range(base, base + 4):
            emit_scores(0, i)

    for g in range(NT // 8):
        ln_group(x, g, nxT, act_copies=(g == 0),
                 half_hook=lambda half, g=g: kv_half(g, half))

    # block-1 queries after all x LN so they never delay kT chunks
    ln_group(xq, 1, nxqT)
    emit_qproj(1)
    # interleave block-1 scores with: block-0 A halves (exp(0) fully drained
    # once ACT reaches exp(1)), then block-0 B halves, then block-1 A halves
    for i in range(NT):
        emit_scores(1, i)
        if i < 8:
            emit_attnv_half_a(0, i)
        elif i < 24:
            if i % 2 == 0:
                emit_attnv_half_b(0, (i - 8) // 2)
        else:
            emit_attnv_half_a(1, i - 24)
    for j in range(QBLK // P):
        emit_attnv_half_b(1, j)


_CACHE = {}


def _patch_act_tables():
    # Force every activation onto the natural_log_exp_and_others set (it has
    # both Ln and Exp, the only functions this kernel uses). The default
    # chooser puts Ln and Exp in different sets, and LN interleaved with the
    # softmax exp stream then reloads tables (~2.7us) on every switch.
    # Emptying the other sets preserves dict order, so act_func_set_id
    # indices stay aligned with act_info.json.
    if getattr(bacc, "_act_tables_patched", False):
        return
    orig = bacc.get_activation_tables

    def patched(module_arch):
        tabs = orig(module_arch)
        keep = "natural_log_exp_and_others"
        if keep in tabs:
            tabs = {k: (v if k == keep else type(v)()) for k, v in tabs.items()}
        return tabs

    bacc.get_activation_tables = patched
    bacc._act_tables_patched = True


def _build():
    if "nc" in _CACHE:
        return _CACHE["nc"]
    _patch_act_tables()
    nc = bacc.Bacc("TRN2", target_bir_lowering=False, debug=False, num_devices=8)
    aps = {}
    for name, shape in (
        ("x", [S, C]), ("xq", [SQ, C]),
        ("wq", [C, C]), ("wk", [C, C]), ("wv", [C, C]), ("wo", [C, C]),
        ("bq", [C, 1]), ("bob", [P, C]), ("ident", [P, P]),
    ):
        aps[name] = nc.dram_tensor(name, shape, F32, kind="ExternalInput").ap()
    aps["out"] = nc.dram_tensor("out", [SQ, C], F32, kind="ExternalOutput").ap()
    with tile.TileContext(nc) as tc:
        _attention_kernel(tc, aps)
    nc.compile()
    _CACHE["nc"] = nc
    return nc


def _host_fold(gamma, beta, Wq, bq, Wk, bk, Wv, bv, Wo, bo):
    scale = 1.0 / np.sqrt(np.float32(C))
    f = {}
    f["wq"] = (gamma[:, None] * Wq * scale).astype(np.float32)
    f["bq"] = ((beta @ Wq + bq) * scale).astype(np.float32).reshape(C, 1)
    f["wk"] = (gamma[:, None] * Wk).astype(np.float32)
    f["wv"] = (gamma[:, None] * Wv).astype(np.float32)
    # v bias (incl. beta@Wv) passes through softmax untouched; fold via Wo.
    bvf = (beta @ Wv + bv).astype(np.float32)
    f["wo"] = np.asarray(Wo, dtype=np.float32)
    bof = (np.asarray(bo, np.float32) + bvf @ np.asarray(Wo, np.float32))
    f["bob"] = np.ascontiguousarray(np.broadcast_to(bof, (P, C)))
    f["ident"] = np.eye(P, dtype=np.float32)
    return f


def make_in_maps(x, gamma, beta, Wq, bq, Wk, bk, Wv, bv, Wo, bo):
    x = np.asarray(x, dtype=np.float32)
    folded = _host_fold(
        np.asarray(gamma, np.float32), np.asarray(beta, np.float32),
        np.asarray(Wq, np.float32), np.asarray(bq, np.float32),
        np.asarray(Wk, np.float32), np.asarray(bk, np.float32),
        np.asarray(Wv, np.float32), np.asarray(bv, np.float32),
        np.asarray(Wo, np.float32), np.asarray(bo, np.float32))
    xs = x.reshape(B, S, C)
    in_maps = []
    for core in range(8):
        bi, half = core // 2, core % 2
        m = dict(folded)
        m["x"] = np.ascontiguousarray(xs[bi])
        m["xq"] = np.ascontiguousarray(xs[bi, half * SQ:(half + 1) * SQ])
        in_maps.append(m)
    return in_maps


def assemble(results):
    full = np.empty((B, S, C), dtype=np.float32)
    for core in range(8):
        bi, half = core // 2, core % 2
        full[bi, half * SQ:(half + 1) * SQ] = results[core]["out"]
    return full.reshape(B, HH, WW, C)


def kernel(x, gamma, beta, Wq, bq, Wk, bk, Wv, bv, Wo, bo):
    nc = _build()
    in_maps = make_in_maps(x, gamma, beta, Wq, bq, Wk, bk, Wv, bv, Wo, bo)
    res = run_bass_kernel_spmd(nc, in_maps, list(range(8)))
    return assemble(res.results)

